# revision 24
# baseline (speedup 1.0000x reference)
"""B3-spline undecimated wavelet transform (a-trous, 3 levels) on 8 trn2 cores.

kernel(x: [16, 1024, 1024] f32) -> [16, 4, 1024, 1024] f32  ([w1, w2, w3, c3])

Sharding: pure data parallel, batch 16 -> 2 images per NeuronCore.

Per-core kernel: each level's separable dilated 5x5 B3 smoothing is fused
into 5 PSUM-accumulated banded matmuls on the tensor engine:
    y'[h, w] = sum_k W5[k] * (A_d @ y)[h, w + (k-2)*d]
A_d is the banded H-conv matrix with reflect padding folded into top/bottom
blocks; the W-shift is a free-axis offset on the rhs AP; W-reflect comes from
8 mirrored pad columns in SBUF. H uses overlapping 128-row tiles (stride 112)
so each output tile is one K=128 window -> one matmul per tap.

Scheduling: everything is tile-granular so DMA streams continuously instead
of in end-of-level bursts. Queue roles: SP HWDGE ring = per-tile input loads
+ inter-tile seam copies + even-tile output flushes; ACT HWDGE ring = const
loads + PSUM->SBUF evacuation copies; SWDGE (gpsimd) = odd-tile output
flushes. Details w_j = y_{j-1} - y_j run on DVE reading PSUM directly, so
they don't serialize behind the ACT evacuation.
"""
import sys
sys.path.insert(0, "/opt/trn_rl_repo")
import contextlib
import numpy as np
import concourse.bass as bass
import concourse.mybir as mybir
from concourse import bacc
from concourse.tile import TileContext

DT = mybir.dt
F32 = DT.float32
F32R = DT.float32r

H = W = 1024
PAD = 8
PW = W + 2 * PAD
NT = 9
STRIDE = 112
DILS = (1, 2, 4)
W5 = np.array([1.0, 4.0, 6.0, 4.0, 1.0]) / 16.0
TAP_ORDER = (0, 4, 1, 3, 2)
SCALE_OF_TAP = {0: 0, 4: 0, 1: 1, 3: 1, 2: 2}
SCALES = (1.0 / 16.0, 4.0 / 16.0, 6.0 / 16.0)


def tile_geom(t):
    if t == 0:
        return 0, 120, 0
    if t == NT - 1:
        return STRIDE * t + 8, 120, 8
    return STRIDE * t + 8, 112, 8


def build_A(cls, d):
    _, M, lo = tile_geom({"top": 0, "int": 1, "bot": NT - 1}[cls])
    A = np.zeros((128, 128), np.float64)
    for m in range(M):
        for i in range(5):
            if cls == "int":
                k = m + 8 + (i - 2) * d
            elif cls == "top":
                g = m + (i - 2) * d
                k = -g if g < 0 else g
            else:
                g = 904 + m + (i - 2) * d
                k = (2046 - g if g > 1023 else g) - 896
            A[k, lo + m] += W5[i]
    return A


def build(n_img=2, n_cores=8, reps=1, bench=False):
    nc = bacc.Bacc(trn_type="TRN2", target_bir_lowering=False, debug=False,
                   num_devices=n_cores)
    x_d = nc.dram_tensor("x", [n_img, H, W], F32R, kind="ExternalInput")
    if bench:
        o_d = nc.dram_tensor("o_scratch", [n_img, 4, H, W], F32,
                             kind="Internal")
        dummy_d = nc.dram_tensor("out", [1, 64], F32, kind="ExternalOutput")
    else:
        o_d = nc.dram_tensor("o", [n_img, 4, H, W], F32, kind="ExternalOutput")

    # All 36 banded matrices in one DRAM blob, L0's 12 first so the first
    # level's matmuls aren't gated on the full const load.
    keys = []
    for li in range(len(DILS)):
        for cls in ("top", "int", "bot"):
            for si in range(len(SCALES)):
                keys.append((li, cls, si))
    blob = np.zeros((128, len(keys) * 128), np.float32)
    col_of = {}
    for i, (li, cls, si) in enumerate(keys):
        blob[:, 128 * i:128 * (i + 1)] = (
            build_A(cls, DILS[li]) * SCALES[si]).astype(np.float32)
        col_of[(li, cls, si)] = 128 * i
    blob_d = nc.inline_tensor(blob, name="mats")

    with TileContext(nc) as tc:
        ctx = contextlib.ExitStack()
        with ctx:
            consts = ctx.enter_context(tc.tile_pool(name="consts", bufs=1))
            ypool = ctx.enter_context(tc.tile_pool(name="ybuf", bufs=3))
            psum = ctx.enter_context(tc.tile_pool(name="acc", bufs=8, space="PSUM"))
            wstage = ctx.enter_context(tc.tile_pool(name="wstage", bufs=2))

            mat_sb = consts.tile([128, len(keys) * 128], F32R,
                                 tag="mats", name="mats")
            NL0 = 12 * 128
            nc.scalar.dma_start(out=mat_sb[:, 0:NL0],
                                in_=blob_d.ap().bitcast(F32R)[:, 0:NL0])
            nc.scalar.dma_start(out=mat_sb[:, NL0:],
                                in_=blob_d.ap().bitcast(F32R)[:, NL0:])

            def mat(li, cls, si):
                c = col_of[(li, cls, si)]
                return mat_sb[:, c:c + 128]

            def flush_tile(big, img, ch, t):
                og, M, lo = tile_geom(t)
                eng = nc.gpsimd
                eng.dma_start(out=o_d[img, ch, og:og + M, :],
                              in_=big[lo:lo + M, 1024 * t:1024 * t + 1024])

            def fill_pads(ybig, t):
                b = PW * t
                nc.vector.tensor_copy(ybig[:, b:b + PAD],
                                      ybig[:, b + 2 * PAD:b + PAD:-1])
                nc.vector.tensor_copy(ybig[:, b + W + PAD:b + W + 2 * PAD],
                                      ybig[:, b + W + PAD - 2:b + W - 2:-1])

            def seams(ybig, t):
                b = PW * t
                if t > 0:
                    nc.sync.dma_start(
                        out=ybig[0:8, b + PAD:b + W + PAD],
                        in_=ybig[112:120, b - PW + PAD:b - PW + W + PAD])
                if t < NT - 1:
                    nc.sync.dma_start(
                        out=ybig[120:128, b + PAD:b + W + PAD],
                        in_=ybig[8:16, b + PW + PAD:b + PW + W + PAD])

            def load_img(img):
                ybig = ypool.tile([128, NT * PW], F32R, tag="ybig", name="ybig")
                for t in range(NT):
                    nc.sync.dma_start(
                        out=ybig[:, PW * t + PAD:PW * t + PAD + W],
                        in_=bass.AP(x_d, (img * H + STRIDE * t) * W,
                                    [[W, 128], [1, W]]))
                    fill_pads(ybig, t)
                return ybig

            def level(img, li, ycur):
                d = DILS[li]
                last = (li == len(DILS) - 1)
                ynext = None
                if not last:
                    ynext = ypool.tile([128, NT * PW], F32R, tag="ybig",
                                       name="ynbig")
                wbig = wstage.tile([128, NT * 1024], F32, tag="wbig",
                                   name="wbig")
                cbig = None
                if last:
                    cbig = wstage.tile([128, NT * 1024], F32, tag="wbig",
                                       name="cbig")

                def do_tile(t):
                    og, M, lo = tile_geom(t)
                    cls = "top" if t == 0 else ("bot" if t == NT - 1 else "int")
                    for c in range(2):
                        col = PAD + 512 * c
                        acc = psum.tile([128, 512], F32, tag="acc", name="acc")
                        for j, i in enumerate(TAP_ORDER):
                            sh = PW * t + col + (i - 2) * d
                            nc.tensor.matmul(
                                acc[:],
                                mat(li, cls, SCALE_OF_TAP[i]),
                                ycur[:, sh:sh + 512],
                                start=(j == 0), stop=(j == 4))
                        if not last:
                            nc.scalar.copy(
                                ynext[:, PW * t + col:PW * t + col + 512],
                                acc[:])
                            y1s = ynext[:, PW * t + col:
                                        PW * t + col + 512].bitcast(F32)
                        else:
                            nc.scalar.copy(
                                cbig[:, 1024 * t + 512 * c:1024 * t + 512 * c + 512],
                                acc[:])
                            y1s = cbig[:, 1024 * t + 512 * c:
                                       1024 * t + 512 * c + 512]
                        wslice = wbig[:, 1024 * t + 512 * c:
                                      1024 * t + 512 * c + 512]
                        y0s = ycur[:, PW * t + col:
                                   PW * t + col + 512].bitcast(F32)
                        # subtract reads the evacuated copy, not PSUM: each
                        # acc bank then has a single reader (the evac) and
                        # frees at the Act engine's pace
                        nc.vector.tensor_tensor(
                            wslice, y0s, y1s, mybir.AluOpType.subtract)

                for t in range(NT):
                    do_tile(t)
                    if t >= 1:
                        flush_tile(wbig, img, li, t - 1)
                        if last:
                            flush_tile(cbig, img, 3, t - 1)
                        elif t >= 2:
                            seams(ynext, t - 2)
                            fill_pads(ynext, t - 2)
                flush_tile(wbig, img, li, NT - 1)
                if last:
                    flush_tile(cbig, img, 3, NT - 1)
                else:
                    for t in (NT - 2, NT - 1):
                        seams(ynext, t)
                        fill_pads(ynext, t)
                return ynext

            def run_all():
                # Interleave images at level granularity: the second image's
                # input streams in during the first's L1 compute, so no phase
                # boundary ever waits on an input load (removes the
                # inter-image pipeline bubble).
                if n_img == 2:
                    y0 = load_img(0)
                    y0 = level(0, 0, y0)
                    y1 = load_img(1)
                    y0 = level(0, 1, y0)
                    y1 = level(1, 0, y1)
                    level(0, 2, y0)
                    y1 = level(1, 1, y1)
                    level(1, 2, y1)
                else:
                    for img in range(n_img):
                        y = load_img(img)
                        for li in range(len(DILS)):
                            y = level(img, li, y)

            if bench and reps > 1:
                with tc.For_i(0, reps):
                    run_all()
            else:
                run_all()
            if bench:
                nc.sync.dma_start(out=dummy_d[:], in_=o_d[0, 0, 0:1, 0:64])

    nc.compile()
    return nc


_NC = None


def kernel(x):
    global _NC
    x = np.ascontiguousarray(np.asarray(x), dtype=np.float32)
    B = x.shape[0]
    n_cores = 8
    per = B // n_cores
    if _NC is None:
        _NC = build(n_img=per, n_cores=n_cores)
    from concourse.bass_utils import run_bass_kernel_spmd
    ins = [{"x": np.ascontiguousarray(x[per * c:per * c + per])}
           for c in range(n_cores)]
    res = run_bass_kernel_spmd(_NC, ins, core_ids=list(range(n_cores)))
    return np.concatenate([r["o"] for r in res.results], axis=0)


# revision 29
# speedup vs baseline: 1.0854x; 1.0854x over previous
"""B3-spline undecimated wavelet transform (a-trous, 3 levels) on 8 trn2 cores.

kernel(x: [16, 1024, 1024] f32) -> [16, 4, 1024, 1024] f32  ([w1, w2, w3, c3])

Sharding: pure data parallel, batch 16 -> 2 images per NeuronCore.

Per-core kernel: each level's separable dilated 5x5 B3 smoothing is fused
into 5 PSUM-accumulated banded matmuls on the tensor engine:
    y'[h, w] = sum_k W5[k] * (A_d @ y)[h, w + (k-2)*d]
A_d is the banded H-conv matrix with reflect padding folded into top/bottom
blocks; the W-shift is a free-axis offset on the rhs AP; W-reflect comes from
8 mirrored pad columns in SBUF. H uses overlapping 128-row tiles (stride 112)
so each output tile is one K=128 window -> one matmul per tap.

Scheduling: everything is tile-granular so DMA streams continuously instead
of in end-of-level bursts. Queue roles: SP HWDGE ring = per-tile input loads
+ inter-tile seam copies + even-tile output flushes; ACT HWDGE ring = const
loads + PSUM->SBUF evacuation copies; SWDGE (gpsimd) = odd-tile output
flushes. Details w_j = y_{j-1} - y_j run on DVE reading PSUM directly, so
they don't serialize behind the ACT evacuation.
"""
import sys
sys.path.insert(0, "/opt/trn_rl_repo")
import contextlib
import numpy as np
import concourse.bass as bass
import concourse.mybir as mybir
from concourse import bacc
from concourse.tile import TileContext

DT = mybir.dt
F32 = DT.float32
F32R = DT.float32r

H = W = 1024
PAD = 8
PW = W + 2 * PAD
NT = 9
STRIDE = 112
DILS = (1, 2, 4)
W5 = np.array([1.0, 4.0, 6.0, 4.0, 1.0]) / 16.0
TAP_ORDER = (0, 4, 1, 3, 2)
SCALE_OF_TAP = {0: 0, 4: 0, 1: 1, 3: 1, 2: 2}
SCALES = (1.0 / 16.0, 4.0 / 16.0, 6.0 / 16.0)


def tile_geom(t):
    if t == 0:
        return 0, 120, 0
    if t == NT - 1:
        return STRIDE * t + 8, 120, 8
    return STRIDE * t + 8, 112, 8


def build_A(cls, d):
    _, M, lo = tile_geom({"top": 0, "int": 1, "bot": NT - 1}[cls])
    A = np.zeros((128, 128), np.float64)
    for m in range(M):
        for i in range(5):
            if cls == "int":
                k = m + 8 + (i - 2) * d
            elif cls == "top":
                g = m + (i - 2) * d
                k = -g if g < 0 else g
            else:
                g = 904 + m + (i - 2) * d
                k = (2046 - g if g > 1023 else g) - 896
            A[k, lo + m] += W5[i]
    return A


def build(n_img=2, n_cores=8, reps=1, bench=False):
    nc = bacc.Bacc(trn_type="TRN2", target_bir_lowering=False, debug=False,
                   num_devices=n_cores)
    x_d = nc.dram_tensor("x", [n_img, H, W], F32R, kind="ExternalInput")
    if bench:
        o_d = nc.dram_tensor("o_scratch", [n_img, 4, H, W], F32,
                             kind="Internal")
        dummy_d = nc.dram_tensor("out", [1, 64], F32, kind="ExternalOutput")
    else:
        o_d = nc.dram_tensor("o", [n_img, 4, H, W], F32, kind="ExternalOutput")

    # All 36 banded matrices in one DRAM blob, L0's 12 first so the first
    # level's matmuls aren't gated on the full const load.
    keys = []
    for li in range(len(DILS)):
        for cls in ("top", "int", "bot"):
            for si in range(len(SCALES)):
                keys.append((li, cls, si))
    blob = np.zeros((128, len(keys) * 128), np.float32)
    col_of = {}
    for i, (li, cls, si) in enumerate(keys):
        blob[:, 128 * i:128 * (i + 1)] = (
            build_A(cls, DILS[li]) * SCALES[si]).astype(np.float32)
        col_of[(li, cls, si)] = 128 * i
    blob_d = nc.inline_tensor(blob, name="mats")

    with TileContext(nc) as tc:
        ctx = contextlib.ExitStack()
        with ctx:
            consts = ctx.enter_context(tc.tile_pool(name="consts", bufs=1))
            ypool = ctx.enter_context(tc.tile_pool(name="ybuf", bufs=3))
            psum = ctx.enter_context(tc.tile_pool(name="acc", bufs=8, space="PSUM"))
            wstage = ctx.enter_context(tc.tile_pool(name="wstage", bufs=12))

            mat_sb = consts.tile([128, len(keys) * 128], F32R,
                                 tag="mats", name="mats")
            NL0 = 12 * 128
            nc.scalar.dma_start(out=mat_sb[:, 0:NL0],
                                in_=blob_d.ap().bitcast(F32R)[:, 0:NL0])
            nc.scalar.dma_start(out=mat_sb[:, NL0:],
                                in_=blob_d.ap().bitcast(F32R)[:, NL0:])

            def mat(li, cls, si):
                c = col_of[(li, cls, si)]
                return mat_sb[:, c:c + 128]

            def flush_tile(wt, img, ch, t):
                og, M, lo = tile_geom(t)
                nc.gpsimd.dma_start(out=o_d[img, ch, og:og + M, :],
                                    in_=wt[lo:lo + M, :])

            def fill_pads(ybig, t):
                b = PW * t
                nc.vector.tensor_copy(ybig[:, b:b + PAD],
                                      ybig[:, b + 2 * PAD:b + PAD:-1])
                nc.vector.tensor_copy(ybig[:, b + W + PAD:b + W + 2 * PAD],
                                      ybig[:, b + W + PAD - 2:b + W - 2:-1])

            def seams(ybig, t):
                b = PW * t
                if t > 0:
                    nc.sync.dma_start(
                        out=ybig[0:8, b + PAD:b + W + PAD],
                        in_=ybig[112:120, b - PW + PAD:b - PW + W + PAD])
                if t < NT - 1:
                    nc.sync.dma_start(
                        out=ybig[120:128, b + PAD:b + W + PAD],
                        in_=ybig[8:16, b + PW + PAD:b + PW + W + PAD])

            def load_img(img):
                ybig = ypool.tile([128, NT * PW], F32R, tag="ybig", name="ybig")
                for t in range(NT):
                    nc.sync.dma_start(
                        out=ybig[:, PW * t + PAD:PW * t + PAD + W],
                        in_=bass.AP(x_d, (img * H + STRIDE * t) * W,
                                    [[W, 128], [1, W]]))
                    fill_pads(ybig, t)
                return ybig

            def level(img, li, ycur):
                d = DILS[li]
                last = (li == len(DILS) - 1)
                ynext = None
                if not last:
                    ynext = ypool.tile([128, NT * PW], F32R, tag="ybig",
                                       name="ynbig")

                def do_tile(t):
                    og, M, lo = tile_geom(t)
                    cls = "top" if t == 0 else ("bot" if t == NT - 1 else "int")
                    # per-tile staging: the flush DMA reads [lo:lo+M, :] and
                    # the 12-deep pool recycles on a per-tile basis
                    wt = wstage.tile([128, 1024], F32, tag="wt", name="wt")
                    ct = None
                    if last:
                        ct = wstage.tile([128, 1024], F32, tag="wt", name="ct")
                    for c in range(2):
                        col = PAD + 512 * c
                        acc = psum.tile([128, 512], F32, tag="acc", name="acc")
                        for j, i in enumerate(TAP_ORDER):
                            sh = PW * t + col + (i - 2) * d
                            nc.tensor.matmul(
                                acc[:],
                                mat(li, cls, SCALE_OF_TAP[i]),
                                ycur[:, sh:sh + 512],
                                start=(j == 0), stop=(j == 4))
                        if not last:
                            nc.scalar.copy(
                                ynext[:, PW * t + col:PW * t + col + 512],
                                acc[:])
                            y1s = ynext[:, PW * t + col:
                                        PW * t + col + 512].bitcast(F32)
                        else:
                            nc.scalar.copy(ct[:, 512 * c:512 * c + 512],
                                           acc[:])
                            y1s = ct[:, 512 * c:512 * c + 512]
                        y0s = ycur[:, PW * t + col:
                                   PW * t + col + 512].bitcast(F32)
                        # subtract reads the evacuated copy, not PSUM: each
                        # acc bank then has a single reader (the evac) and
                        # frees at the Act engine's pace
                        nc.vector.tensor_tensor(
                            wt[:, 512 * c:512 * c + 512], y0s, y1s,
                            mybir.AluOpType.subtract)
                    flush_tile(wt, img, li, t)
                    if last:
                        flush_tile(ct, img, 3, t)

                for t in range(NT):
                    do_tile(t)
                    if not last:
                        if t >= 2:
                            seams(ynext, t - 2)
                            fill_pads(ynext, t - 2)
                if not last:
                    for t in (NT - 2, NT - 1):
                        seams(ynext, t)
                        fill_pads(ynext, t)
                return ynext

            def run_all():
                # Interleave images at level granularity: the second image's
                # input streams in during the first's L1 compute, so no phase
                # boundary ever waits on an input load (removes the
                # inter-image pipeline bubble).
                if n_img == 2:
                    y0 = load_img(0)
                    y0 = level(0, 0, y0)
                    y1 = load_img(1)
                    y0 = level(0, 1, y0)
                    y1 = level(1, 0, y1)
                    level(0, 2, y0)
                    y1 = level(1, 1, y1)
                    level(1, 2, y1)
                else:
                    for img in range(n_img):
                        y = load_img(img)
                        for li in range(len(DILS)):
                            y = level(img, li, y)

            if bench and reps > 1:
                with tc.For_i(0, reps):
                    run_all()
            else:
                run_all()
            if bench:
                nc.sync.dma_start(out=dummy_d[:], in_=o_d[0, 0, 0:1, 0:64])

    nc.compile()
    return nc


_NC = None


def kernel(x):
    global _NC
    x = np.ascontiguousarray(np.asarray(x), dtype=np.float32)
    B = x.shape[0]
    n_cores = 8
    per = B // n_cores
    if _NC is None:
        _NC = build(n_img=per, n_cores=n_cores)
    from concourse.bass_utils import run_bass_kernel_spmd
    ins = [{"x": np.ascontiguousarray(x[per * c:per * c + per])}
           for c in range(n_cores)]
    res = run_bass_kernel_spmd(_NC, ins, core_ids=list(range(n_cores)))
    return np.concatenate([r["o"] for r in res.results], axis=0)
